# revision 1
# baseline (speedup 1.0000x reference)
"""Trainium2 Bass kernel: single-channel Conv2d.

  x: [32, 224, 224] f32, kernels: [64, 7, 7] f32
  out[b, k, i, j] = sum_{di,dj} x[b, i+di, j+dj] * kernels[k, di, dj]
  -> [32, 64, 218, 218]

Sharding: data-parallel over batch, 4 images per NeuronCore across 8 cores.

Per-core algorithm (fp32r matmuls: fp32 rounded to 11-bit mantissa, which
streams at full PE rate):
  - 4 images = 2 image-pairs. An image-pair's rows are staged in SBUF as
    x2s[row, seg*464 + img*224 + j] (two row-segments 0..127 / 120..223 with
    an 8-row halo, 448 data cols + zero pad per segment).
  - VectorE builds a shift-expanded fp32r copy
        x2g[row, seg*1824 + g*456 + c] = x2s[row, seg*464 + c + g], g=0..3
    (4 column-shifted copies along the free dim; also applies f32r rounding).
  - For each output-row-pair (i, i+1), ONE rectangular SBUF->SBUF DMA
    gathers the patch tile pt[32, 456]:
        pt[dr*4 + g, c] = x2g[i + dr, seg_off + g*456 + c]
    (out is a contiguous [32, 456] tile; in is a plain [8, 1824] slice).
  - Two accumulating matmuls (tap groups d=0,4) with banded 32x128
    stationary weights (precomputed on host, rounded to f32r on device)
    produce a full PSUM tile [128 = 2 rows x 64 ch, 448 = 2 imgs x 224]:
        W[d][dr*4+g, s*64+k] = w[k, dr-s, g+d]   (stream offset d applies
        taps dj = g+d; out-of-band entries are zero).
  - PSUM is evacuated by VectorE+ScalarE into a 16-pair SBUF chunk, which
    is stored with 4 large DMAs (s x img).
  - DMA issue is split between the SP (HWDGE) and Pool (SWDGE) queues.
"""
import sys

sys.path.insert(0, "/opt/trn_rl_repo")

import numpy as np

B, H, W = 32, 224, 224
KCH, KS = 64, 7
HO = WO = H - KS + 1  # 218
NCORES = 8
BLOC = B // NCORES    # 4 images per core
NPAIRS = HO // 2      # 109 output-row-pairs per image-pair

SEGW = 464            # x2s per-segment span (448 data + 16 zero pad)
X2SF = 2 * SEGW       # 928
GSP = 456             # x2g per-shift span (= pt free size)
NG = 4                # shift groups
X2GF = 2 * NG * GSP   # 3648
PTW = GSP             # 456
NST = 448             # matmul stream length (2 imgs x 224)
DVE_COLS = 280        # PSUM evacuation split: VectorE cols, rest ScalarE
CH = 16               # row-pairs per output SBUF chunk
OIMG = KCH * HO * WO

_NC_CACHE = {}


def make_weight_band(kernels: np.ndarray) -> np.ndarray:
    """Banded stationary matrices [2, 32, 128]: index dd covers taps
    dj = g + 4*dd.  W[dd][dr*4 + g, s*64 + k] = kernels[k, dr-s, g+4*dd]."""
    wb = np.zeros((2, 32, 128), dtype=np.float32)
    for dd in range(2):
        d = 4 * dd
        for dr in range(8):
            for g in range(NG):
                dj = g + d
                if dj > KS - 1:
                    continue
                p = dr * 4 + g
                for s in range(2):
                    di = dr - s
                    if 0 <= di < KS:
                        wb[dd, p, s * KCH: (s + 1) * KCH] = kernels[:, di, dj]
    return wb


def _build_nc(iters: int = 1, no_stores: bool = False, no_evac: bool = False,
              no_mm: bool = False, stores_only: bool = False, ch: int = CH):
    import concourse.bacc as bacc
    import concourse.mybir as mybir
    import concourse.tile as tile
    from concourse.bass_types import AP

    F32 = mybir.dt.float32
    F32R = mybir.dt.float32r

    nc = bacc.Bacc("TRN2", target_bir_lowering=False, debug=False,
                   num_devices=NCORES)
    x_d = nc.dram_tensor("x", [BLOC, H, W], F32, kind="ExternalInput").ap()
    wb_d = nc.dram_tensor("wband", [2, 32, 128], F32,
                          kind="ExternalInput").ap()
    out_d = nc.dram_tensor("out", [BLOC, KCH, HO, WO], F32,
                           kind="ExternalOutput").ap()

    with tile.TileContext(nc) as tc:
        with (
            tc.tile_pool(name="wpool", bufs=1) as wpool,
            tc.tile_pool(name="x2pool", bufs=2) as x2pool,
            tc.tile_pool(name="ptpool", bufs=8) as ptpool,
            tc.tile_pool(name="opool", bufs=3) as opool,
            tc.tile_pool(name="psum", bufs=8, space="PSUM") as psum,
        ):
            # ---- stationary weights: [32, 2*128] f32r ----
            wb32 = wpool.tile([32, 2 * 128], F32)
            nc.sync.dma_start(out=wb32[:],
                              in_=wb_d.rearrange("i p m -> p i m"))
            wbr = wpool.tile([32, 2 * 128], F32R)
            nc.vector.tensor_copy(out=wbr[:], in_=wb32[:])

            def body():
                for q in range(2):
                    x2s = x2pool.tile([128, X2SF], F32, tag="x2s")
                    nc.gpsimd.memset(x2s[:], 0.0)
                    for seg in range(2):
                        r_lo = 0 if seg == 0 else 120
                        nrows = 128 if seg == 0 else H - 120
                        nc.sync.dma_start(
                            out=x2s[0:nrows, seg * SEGW: seg * SEGW + 2 * W]
                            .rearrange("r (b j) -> r b j", b=2),
                            in_=x_d[2 * q: 2 * q + 2, r_lo: r_lo + nrows, :]
                            .rearrange("b r j -> r b j"),
                        )
                    # shift-expanded f32r copy
                    x2g = x2pool.tile([128, X2GF], F32R, tag="x2g")
                    for seg in range(2):
                        for g in range(NG):
                            nc.vector.tensor_copy(
                                out=x2g[:, (seg * NG + g) * GSP:
                                        (seg * NG + g + 1) * GSP],
                                in_=x2s[:, seg * SEGW + g:
                                        seg * SEGW + g + GSP],
                            )

                    chunk = None
                    npl = 0
                    chunk_start = 0
                    for pr in range(NPAIRS):
                        i = 2 * pr
                        if pr % ch == 0:
                            npl = min(ch, NPAIRS - pr)
                            chunk = opool.tile([128, ch * NST], F32,
                                               tag="osb")
                            chunk_start = pr
                        seg = 0 if i + 7 <= 127 else 1
                        r0 = i - 120 * seg
                        goff = seg * NG * GSP
                        pt = ptpool.tile([32, PTW], F32R, tag="pt")
                        if not stores_only:
                            dma_eng = nc.sync if pr % 2 == 0 else nc.scalar
                            dma_eng.dma_start(
                                out=pt[:],
                                in_=x2g[r0: r0 + 8, goff: goff + NG * GSP],
                            )
                        pl = pr - chunk_start
                        ps = psum.tile([128, NST], F32)
                        if not (no_mm or stores_only):
                            for dd in range(2):
                                d = 4 * dd
                                nc.tensor.matmul(
                                    out=ps[:],
                                    lhsT=wbr[:, dd * 128: (dd + 1) * 128],
                                    rhs=pt[:, d: d + NST],
                                    start=(dd == 0), stop=(dd == 1),
                                )
                        if not no_evac and not no_mm and not stores_only:
                            nc.vector.tensor_copy(
                                out=chunk[:, pl * NST: pl * NST + DVE_COLS],
                                in_=ps[:, 0:DVE_COLS])
                            nc.scalar.copy(
                                out=chunk[:, pl * NST + DVE_COLS:
                                          (pl + 1) * NST],
                                in_=ps[:, DVE_COLS:NST])
                        if no_stores or no_evac or no_mm:
                            continue
                        if pl == npl - 1:
                            F = ch * NST
                            st_engines = (nc.sync, nc.gpsimd)
                            nst = 0
                            for s in range(2):
                                for img in range(2):
                                    for kh in range(2):  # k-halves
                                        kw = KCH // 2
                                        st_in = AP(
                                            tensor=chunk[:].tensor,
                                            offset=chunk[:].offset
                                            + (s * KCH + kh * kw) * F
                                            + img * W,
                                            ap=((F, kw), (NST, npl),
                                                (1, WO)),
                                        )
                                        st_out = AP(
                                            tensor=out_d.tensor,
                                            offset=(2 * q + img) * OIMG
                                            + kh * kw * HO * WO
                                            + (2 * chunk_start + s) * WO,
                                            ap=((HO * WO, kw), (2 * WO, npl),
                                                (1, WO)),
                                        )
                                        st_engines[nst % 2].dma_start(
                                            out=st_out, in_=st_in)
                                        nst += 1

            if iters == 1:
                body()
            else:
                with tc.For_i(0, iters, 1):
                    body()
    nc.compile()
    return nc


def _get_nc(iters: int = 1, **kw):
    key = (iters, tuple(sorted(kw.items())))
    if key not in _NC_CACHE:
        _NC_CACHE[key] = _build_nc(iters, **kw)
    return _NC_CACHE[key]


def kernel(x: np.ndarray, kernels: np.ndarray) -> np.ndarray:
    from concourse.bass_utils import run_bass_kernel_spmd

    x = np.ascontiguousarray(np.asarray(x, dtype=np.float32))
    kernels = np.ascontiguousarray(np.asarray(kernels, dtype=np.float32))
    wb = make_weight_band(kernels)
    nc = _get_nc()
    in_maps = [
        {"x": x[c * BLOC: (c + 1) * BLOC], "wband": wb}
        for c in range(NCORES)
    ]
    res = run_bass_kernel_spmd(nc, in_maps, core_ids=list(range(NCORES)))
    return np.concatenate([res.results[c]["out"] for c in range(NCORES)],
                          axis=0)



# revision 3
# speedup vs baseline: 77928.4009x; 77928.4009x over previous
"""Trainium2 Bass kernel: single-channel Conv2d.

  x: [32, 224, 224] f32, kernels: [64, 7, 7] f32
  out[b, k, i, j] = sum_{di,dj} x[b, i+di, j+dj] * kernels[k, di, dj]
  -> [32, 64, 218, 218]

Sharding: data-parallel over batch, 4 images (= 2 image-pairs q) per core.

Layout choice (v3): matmul OUTPUT partitions are (img, k) = 2*64 = 128 and
the stream is a single output row j = 0..223.  This makes every output
store DMA fully contiguous per (img, k) in DRAM (13-24 KB descriptors),
which is what the baseline lacked (872 B descriptors -> descriptor-
overhead-bound DMA at ~120 GB/s).

  - Host pre-builds xg[q, seg, p, (img*4+g)*232 + j] = x[2q+img, rlo+p, j+g]
    (4 column-shifted copies, zero-padded), rlo = {0, 96}; and the banded
    stationary weights w3h[128, (r*2+dd)*128 + (img*64+k)]
      row c = img'*64 + dr*4 + g ->  delta(img'==img) * w[k, dr-r, g+4*dd]
    (zero outside 0<=dr-r<7, g+4*dd<7).  Both ship as float32 bits into
    float32r DRAM tensors (f32r = fp32 bits, PE rounds mantissa -> full
    PE rate).
  - Per block of 10 output rows (r0 = 10b, last block r0=208 partial): one
    [64, 232] gather DMA per img from the seg tile (16 rows x 4 shifts),
    then per row r: 2 accumulating matmuls (dd = 0,1; taps dj = g+4*dd)
    with stationary w3[r*2+dd] -> PSUM [128 = 2 img x 64 ch, 224].
  - VectorE (cols 0:138) + ScalarE (cols 138:218) evacuate PSUM into a
    28-row chunk [128, 28*218]; one store DMA per chunk:
      SBUF ((6104,128),(1,nrows*218)) -> DRAM ((OIMG,2),(HO*WO,64),(1,nrows*218))
    i.e. per (img,k) a single contiguous nrows*218-float run.
  - Queues: all loads + gathers on SP (qSyncDynamicHW); all stores on ACT
    (qScalarDynamicHW); no SWDGE (avoids the DVE/GpSimd shared-port trap).
"""
import sys

sys.path.insert(0, "/opt/trn_rl_repo")

import numpy as np

B, H, W = 32, 224, 224
KCH, KS = 64, 7
HO = WO = H - KS + 1  # 218
NCORES = 8
BLOC = B // NCORES    # 4 images per core
NQ = BLOC // 2        # 2 image-pairs per core
RB = 16               # x-rows per gather block
NRB = 10              # output rows per full block
NBLK = 22             # blocks per image-pair (21 full + 1 partial)
XG_G = 232            # xg span per (img, g)
XG_IMG = 4 * XG_G     # 928
XGF = 2 * XG_IMG      # 1856 per seg
PTW = XG_G            # 232
NST = 224             # matmul stream length
DVE_COLS = 138        # PSUM evacuation split: VectorE cols, rest ScalarE
CROWS = 28            # rows per output SBUF chunk
OIMG = KCH * HO * WO
HOWO = HO * WO
SEG_RLO = (0, 96)     # seg -> first x row

_NC_CACHE = {}


def make_w3(kernels: np.ndarray) -> np.ndarray:
    """Banded stationary matrices [128, 20*128].
    w3h[img*64 + dr*4 + g, (r*2+dd)*128 + img*64 + k] = kernels[k, dr-r, g+4dd]
    for 0 <= dr-r < 7 and g+4dd < 7, else 0."""
    w3 = np.zeros((10, 2, 128, 128), dtype=np.float32)
    for r in range(10):
        for dd in range(2):
            for dr in range(RB):
                di = dr - r
                if not (0 <= di < KS):
                    continue
                for g in range(4):
                    dj = g + 4 * dd
                    if dj >= KS:
                        continue
                    for img in range(2):
                        w3[r, dd, img * 64 + dr * 4 + g,
                           img * 64: img * 64 + KCH] = kernels[:, di, dj]
    # -> [128, 20*128] with free = (r, dd, p)
    return np.ascontiguousarray(
        w3.transpose(2, 0, 1, 3).reshape(128, 20 * 128))


def make_xg(xc: np.ndarray) -> np.ndarray:
    """Shift-expanded input [NQ, 2, 128, XGF] for one core's xc [4, H, W].
    xg[q, seg, p, (img*4+g)*232 + j] = xc[2q+img, SEG_RLO[seg]+p, j+g]."""
    xg = np.zeros((NQ, 2, 128, XGF), dtype=np.float32)
    for seg in range(2):
        rlo = SEG_RLO[seg]
        rows = xc[:, rlo: rlo + 128, :]          # [4, 128, 224]
        for img in range(2):
            for g in range(4):
                base = (img * 4 + g) * XG_G
                xg[:, seg, :, base: base + W - g] = \
                    rows[img::2, :, g:]          # [NQ, 128, 224-g]
    return xg


def _build_nc():
    import concourse.bacc as bacc
    import concourse.mybir as mybir
    import concourse.tile as tile
    from concourse.bass_types import AP

    F32 = mybir.dt.float32
    F32R = mybir.dt.float32r

    nc = bacc.Bacc("TRN2", target_bir_lowering=False, debug=False,
                   num_devices=NCORES)
    xg_d = nc.dram_tensor("xg", [NQ, 2, 128, XGF], F32R,
                          kind="ExternalInput").ap()
    w3_d = nc.dram_tensor("w3", [128, 20 * 128], F32R,
                          kind="ExternalInput").ap()
    out_d = nc.dram_tensor("out", [BLOC, KCH, HO, WO], F32,
                           kind="ExternalOutput").ap()

    with tile.TileContext(nc) as tc:
        with (
            tc.tile_pool(name="wpool", bufs=1) as wpool,
            tc.tile_pool(name="xgpool", bufs=3) as xgpool,
            tc.tile_pool(name="ptpool", bufs=6) as ptpool,
            tc.tile_pool(name="opool", bufs=3) as opool,
            tc.tile_pool(name="psum", bufs=8, space="PSUM") as psum,
        ):
            wfr = wpool.tile([128, 20 * 128], F32R)
            nc.sync.dma_start(out=wfr[:], in_=w3_d)

            for q in range(NQ):
                x2g = []
                for seg in range(2):
                    xt = xgpool.tile([128, XGF], F32R, tag="xg")
                    nc.sync.dma_start(out=xt[:], in_=xg_d[q, seg])
                    x2g.append(xt)

                gi = 0  # global output row within this q
                chunk = None
                crow0 = 0
                nrows_c = 0
                for b in range(NBLK):
                    r0 = NRB * b if b < NBLK - 1 else 208
                    seg = 0 if b <= 11 else 1
                    rb = r0 - SEG_RLO[seg]
                    xt = x2g[seg]
                    pt = ptpool.tile([128, PTW], F32R, tag="pt")
                    for img in range(2):
                        nc.sync.dma_start(
                            out=pt[img * 64: (img + 1) * 64, :],
                            in_=xt[rb: rb + RB,
                                   img * XG_IMG: (img + 1) * XG_IMG]
                            .rearrange("r (g j) -> r g j", g=4),
                        )
                    rlist = range(NRB) if b < NBLK - 1 else range(2, NRB)
                    for r in rlist:
                        ps = psum.tile([128, 512], F32, tag="ps")
                        for dd in range(2):
                            t = (r * 2 + dd) * 128
                            nc.tensor.matmul(
                                out=ps[:, 0:NST],
                                lhsT=wfr[:, t: t + 128],
                                rhs=pt[:, 4 * dd: 4 * dd + NST],
                                start=(dd == 0), stop=(dd == 1),
                            )
                        slot = gi % CROWS
                        if slot == 0:
                            crow0 = gi
                            nrows_c = min(CROWS, HO - crow0)
                            chunk = opool.tile([128, CROWS * WO], F32,
                                               tag="osb")
                        nc.vector.tensor_copy(
                            out=chunk[:, slot * WO: slot * WO + DVE_COLS],
                            in_=ps[:, 0:DVE_COLS])
                        nc.scalar.copy(
                            out=chunk[:, slot * WO + DVE_COLS:
                                      (slot + 1) * WO],
                            in_=ps[:, DVE_COLS:WO])
                        if slot == nrows_c - 1:
                            st_in = AP(
                                tensor=chunk[:].tensor,
                                offset=chunk[:].offset,
                                ap=((CROWS * WO, 128), (1, nrows_c * WO)),
                            )
                            st_out = AP(
                                tensor=out_d.tensor,
                                offset=2 * q * OIMG + crow0 * WO,
                                ap=((OIMG, 2), (HOWO, KCH),
                                    (1, nrows_c * WO)),
                            )
                            nc.scalar.dma_start(out=st_out, in_=st_in)
                        gi += 1
    nc.compile()
    return nc


def _get_nc():
    if "nc" not in _NC_CACHE:
        _NC_CACHE["nc"] = _build_nc()
    return _NC_CACHE["nc"]


def _run(x: np.ndarray, kernels: np.ndarray, **kw):
    from concourse.bass_utils import run_bass_kernel_spmd

    x = np.ascontiguousarray(np.asarray(x, dtype=np.float32))
    kernels = np.ascontiguousarray(np.asarray(kernels, dtype=np.float32))
    w3h = make_w3(kernels)
    nc = _get_nc()
    in_maps = [
        {"xg": make_xg(x[c * BLOC: (c + 1) * BLOC]), "w3": w3h}
        for c in range(NCORES)
    ]
    return run_bass_kernel_spmd(nc, in_maps, core_ids=list(range(NCORES)),
                                **kw)


def kernel(x: np.ndarray, kernels: np.ndarray) -> np.ndarray:
    res = _run(x, kernels)
    return np.concatenate([res.results[c]["out"] for c in range(NCORES)],
                          axis=0)


# revision 4
# speedup vs baseline: 86140.3981x; 1.1054x over previous
"""Trainium2 Bass kernel: single-channel Conv2d.

  x: [32, 224, 224] f32, kernels: [64, 7, 7] f32
  out[b, k, i, j] = sum_{di,dj} x[b, i+di, j+dj] * kernels[k, di, dj]
  -> [32, 64, 218, 218]

Sharding: data-parallel over batch, 4 images (= 2 image-pairs q) per core.

Layout (v4): matmul OUTPUT partitions are (img, k) = 2*64 = 128 and the
stream is a single output row j = 0..223, so every store DMA is fully
contiguous per (img, k) in DRAM (nrows*218-float runs, 3.5-24 KB
descriptors; the v1 baseline's 872 B descriptors made DMA descriptor-
overhead-bound at ~120 GB/s).

  - Host ships x re-laid-out as xs[q, seg, 2*p+img, j] = x[2q+img,
    rlo[seg]+p, j]  (5 segs of 64 rows per image-pair, images interleaved
    in partitions so a 16-row gather read spans 8 DMA ports instead of 4),
    zero-padded to 240 cols; and banded stationary weights
    w3h[128, (r*2+dd)*128 + img*64 + k]:
      row c = img'*64 + dr*4 + g -> delta(img'==img) * w[k, dr-r, g+4*dd]
    (zero outside 0<=dr-r<7, g+4*dd<7).  Both ship as raw fp32 bits into
    float32r tensors (f32r = fp32 bits; PE rounds the mantissa, streams at
    full PE rate).
  - Per block of 10 output rows (r0 = 10b, last block r0=208 partial): one
    [64, 232] gather DMA per img with in-AP ((480,16),(1,4),(1,232)) -- the
    (1,4) dim materializes the 4 column shifts; then per row r: 2
    accumulating matmuls (dd = 0,1; taps dj = g+4*dd) with stationary
    w3[r*2+dd] -> PSUM [128 = 2 img x 64 ch, 224].
  - VectorE (cols 0:138) + ScalarE (cols 138:218) evacuate PSUM into
    chunks of ramped size [4, 8, 16, 28, ...] rows (ramp -> first store
    DMA issues early); one store DMA per chunk:
      SBUF ((6104,128),(1,nrows*218)) -> DRAM ((OIMG,2),(HO*WO,64),(1,nrows*218)).
  - Queues: loads + gathers on SP (qSyncDynamicHW); stores on ACT
    (qScalarDynamicHW); no SWDGE (avoids the DVE/GpSimd shared-port trap).
"""
import sys

sys.path.insert(0, "/opt/trn_rl_repo")

import numpy as np

B, H, W = 32, 224, 224
KCH, KS = 64, 7
HO = WO = H - KS + 1  # 218
NCORES = 8
BLOC = B // NCORES    # 4 images per core
NQ = BLOC // 2        # 2 image-pairs per core
RB = 16               # x-rows per gather block
NRB = 10              # output rows per full block
NBLK = 22             # blocks per image-pair (21 full + 1 partial)
NSEG = 5
SEG_RLO = (0, 48, 96, 144, 160)   # seg -> first x row (64 rows each)
SEG_OF_BLOCK = (0, 0, 0, 0, 0, 1, 1, 1, 1, 1, 2, 2, 2, 2, 2,
                3, 3, 3, 3, 3, 4, 4)
XSW = 240             # xs cols per row (224 + 16 zero pad)
PTW = 232
NST = 224             # matmul stream length
DVE_COLS = 138        # PSUM evacuation split: VectorE cols, rest ScalarE
CROWS = 28            # max rows per output SBUF chunk
CHUNK_SIZES = (4, 8, 16, 28, 28, 28, 28, 28, 28, 22)   # sums to 218
OIMG = KCH * HO * WO
HOWO = HO * WO

_NC_CACHE = {}


def make_w3(kernels: np.ndarray) -> np.ndarray:
    """Banded stationary matrices [128, 20*128].
    w3h[img*64 + dr*4 + g, (r*2+dd)*128 + img*64 + k] = kernels[k, dr-r, g+4dd]
    for 0 <= dr-r < 7 and g+4dd < 7, else 0."""
    w3 = np.zeros((10, 2, 128, 128), dtype=np.float32)
    for r in range(10):
        for dd in range(2):
            for dr in range(RB):
                di = dr - r
                if not (0 <= di < KS):
                    continue
                for g in range(4):
                    dj = g + 4 * dd
                    if dj >= KS:
                        continue
                    for img in range(2):
                        w3[r, dd, img * 64 + dr * 4 + g,
                           img * 64: img * 64 + KCH] = kernels[:, di, dj]
    return np.ascontiguousarray(
        w3.transpose(2, 0, 1, 3).reshape(128, 20 * 128))


def make_xs(xc: np.ndarray) -> np.ndarray:
    """Interleaved input segs [NQ, NSEG, 128, XSW] for one core's xc [4,H,W].
    xs[q, s, 2*p + img, j] = xc[2q+img, SEG_RLO[s]+p, j], zero-padded."""
    xs = np.zeros((NQ, NSEG, 128, XSW), dtype=np.float32)
    for s in range(NSEG):
        rlo = SEG_RLO[s]
        blk = xc[:, rlo: rlo + 64, :]            # [4, 64, 224]
        for img in range(2):
            xs[:, s, img::2, :W] = blk[img::2]   # [NQ, 64, 224]
    return xs


def _build_nc():
    import concourse.bacc as bacc
    import concourse.mybir as mybir
    import concourse.tile as tile
    from concourse.bass_types import AP

    F32 = mybir.dt.float32
    F32R = mybir.dt.float32r

    nc = bacc.Bacc("TRN2", target_bir_lowering=False, debug=False,
                   num_devices=NCORES)
    xs_d = nc.dram_tensor("xs", [NQ, NSEG, 128, XSW], F32R,
                          kind="ExternalInput").ap()
    w3_d = nc.dram_tensor("w3", [128, 20 * 128], F32R,
                          kind="ExternalInput").ap()
    out_d = nc.dram_tensor("out", [BLOC, KCH, HO, WO], F32,
                           kind="ExternalOutput").ap()

    with tile.TileContext(nc) as tc:
        with (
            tc.tile_pool(name="wpool", bufs=1) as wpool,
            tc.tile_pool(name="xspool", bufs=7) as xspool,
            tc.tile_pool(name="ptpool", bufs=6) as ptpool,
            tc.tile_pool(name="opool", bufs=3) as opool,
            tc.tile_pool(name="psum", bufs=8, space="PSUM") as psum,
        ):
            wfr = None
            for q in range(NQ):
                xseg = []
                for s in range(NSEG):
                    xt = xspool.tile([128, XSW], F32R, tag="xs")
                    nc.sync.dma_start(out=xt[:], in_=xs_d[q, s])
                    xseg.append(xt)
                    if q == 0 and s == 0:
                        # W load after the first seg so block 0's gather
                        # isn't stuck behind 1.3 MB of weights.
                        wfr = wpool.tile([128, 20 * 128], F32R)
                        nc.sync.dma_start(out=wfr[:], in_=w3_d)

                gi = 0
                ci = 0          # chunk index within q
                chunk = None
                crow0 = 0
                nrows_c = 0
                for b in range(NBLK):
                    r0 = NRB * b if b < NBLK - 1 else 208
                    s = SEG_OF_BLOCK[b]
                    xt = xseg[s]
                    pb = 2 * (r0 - SEG_RLO[s])   # partition of (row r0, img0)
                    pt = ptpool.tile([128, PTW], F32R, tag="pt")
                    for img in range(2):
                        nc.sync.dma_start(
                            out=pt[img * 64: (img + 1) * 64, :],
                            in_=AP(
                                tensor=xt[:].tensor,
                                offset=xt[:].offset + (pb + img) * XSW,
                                ap=((2 * XSW, RB), (1, 4), (1, PTW)),
                            ),
                        )
                    rlist = range(NRB) if b < NBLK - 1 else range(2, NRB)
                    for r in rlist:
                        ps = psum.tile([128, 512], F32, tag="ps")
                        for dd in range(2):
                            t = (r * 2 + dd) * 128
                            nc.tensor.matmul(
                                out=ps[:, 0:NST],
                                lhsT=wfr[:, t: t + 128],
                                rhs=pt[:, 4 * dd: 4 * dd + NST],
                                start=(dd == 0), stop=(dd == 1),
                            )
                        slot = gi - crow0
                        if chunk is None:
                            nrows_c = CHUNK_SIZES[ci]
                            chunk = opool.tile([128, CROWS * WO], F32,
                                               tag="osb")
                        nc.vector.tensor_copy(
                            out=chunk[:, slot * WO: slot * WO + DVE_COLS],
                            in_=ps[:, 0:DVE_COLS])
                        nc.scalar.copy(
                            out=chunk[:, slot * WO + DVE_COLS:
                                      (slot + 1) * WO],
                            in_=ps[:, DVE_COLS:WO])
                        if slot == nrows_c - 1:
                            st_in = AP(
                                tensor=chunk[:].tensor,
                                offset=chunk[:].offset,
                                ap=((CROWS * WO, 128), (1, nrows_c * WO)),
                            )
                            st_out = AP(
                                tensor=out_d.tensor,
                                offset=2 * q * OIMG + crow0 * WO,
                                ap=((OIMG, 2), (HOWO, KCH),
                                    (1, nrows_c * WO)),
                            )
                            nc.scalar.dma_start(out=st_out, in_=st_in)
                            chunk = None
                            ci += 1
                            crow0 = gi + 1
                        gi += 1
    nc.compile()
    return nc


def _get_nc():
    if "nc" not in _NC_CACHE:
        _NC_CACHE["nc"] = _build_nc()
    return _NC_CACHE["nc"]


def _run(x: np.ndarray, kernels: np.ndarray, **kw):
    from concourse.bass_utils import run_bass_kernel_spmd

    x = np.ascontiguousarray(np.asarray(x, dtype=np.float32))
    kernels = np.ascontiguousarray(np.asarray(kernels, dtype=np.float32))
    w3h = make_w3(kernels)
    nc = _get_nc()
    in_maps = [
        {"xs": make_xs(x[c * BLOC: (c + 1) * BLOC]), "w3": w3h}
        for c in range(NCORES)
    ]
    return run_bass_kernel_spmd(nc, in_maps, core_ids=list(range(NCORES)),
                                **kw)


def kernel(x: np.ndarray, kernels: np.ndarray) -> np.ndarray:
    res = _run(x, kernels)
    return np.concatenate([res.results[c]["out"] for c in range(NCORES)],
                          axis=0)


# revision 7
# speedup vs baseline: 86621.9236x; 1.0056x over previous
"""Trainium2 Bass kernel: single-channel Conv2d.

  x: [32, 224, 224] f32, kernels: [64, 7, 7] f32
  out[b, k, i, j] = sum_{di,dj} x[b, i+di, j+dj] * kernels[k, di, dj]
  -> [32, 64, 218, 218]

Sharding: data-parallel over batch, 4 images (= 2 image-pairs q) per core.

Layout (v4): matmul OUTPUT partitions are (img, k) = 2*64 = 128 and the
stream is a single output row j = 0..223, so every store DMA is fully
contiguous per (img, k) in DRAM (nrows*218-float runs, 3.5-24 KB
descriptors; the v1 baseline's 872 B descriptors made DMA descriptor-
overhead-bound at ~120 GB/s).

  - Host ships x re-laid-out as xs[q, seg, 2*p+img, j] = x[2q+img,
    rlo[seg]+p, j]  (5 segs of 64 rows per image-pair, images interleaved
    in partitions so a 16-row gather read spans 8 DMA ports instead of 4),
    zero-padded to 240 cols; and banded stationary weights
    w3h[128, (r*2+dd)*128 + img*64 + k]:
      row c = img'*64 + dr*4 + g -> delta(img'==img) * w[k, dr-r, g+4*dd]
    (zero outside 0<=dr-r<7, g+4*dd<7).  Both ship as raw fp32 bits into
    float32r tensors (f32r = fp32 bits; PE rounds the mantissa, streams at
    full PE rate).
  - Per block of 10 output rows (r0 = 10b, last block r0=208 partial): one
    [64, 232] gather DMA per img with in-AP ((480,16),(1,4),(1,232)) -- the
    (1,4) dim materializes the 4 column shifts; then per row r: 2
    accumulating matmuls (dd = 0,1; taps dj = g+4*dd) with stationary
    w3[r*2+dd] -> PSUM [128 = 2 img x 64 ch, 224].
  - VectorE (cols 0:138) + ScalarE (cols 138:218) evacuate PSUM into
    chunks of ramped size [4, 8, 16, 28, ...] rows (ramp -> first store
    DMA issues early); one store DMA per chunk:
      SBUF ((6104,128),(1,nrows*218)) -> DRAM ((OIMG,2),(HO*WO,64),(1,nrows*218)).
  - Queues: loads + gathers on SP (qSyncDynamicHW); stores on ACT
    (qScalarDynamicHW); no SWDGE (avoids the DVE/GpSimd shared-port trap).
"""
import sys

sys.path.insert(0, "/opt/trn_rl_repo")

import numpy as np

B, H, W = 32, 224, 224
KCH, KS = 64, 7
HO = WO = H - KS + 1  # 218
NCORES = 8
BLOC = B // NCORES    # 4 images per core
NQ = BLOC // 2        # 2 image-pairs per core
RB = 16               # x-rows per gather block
NRB = 10              # output rows per full block
NBLK = 22             # blocks per image-pair (21 full + 1 partial)
NSEG = 5
SEG_RLO = (0, 48, 96, 144, 160)   # seg -> first x row (64 rows each)
SEG_OF_BLOCK = (0, 0, 0, 0, 0, 1, 1, 1, 1, 1, 2, 2, 2, 2, 2,
                3, 3, 3, 3, 3, 4, 4)
XSW = 240             # xs cols per row (224 + 16 zero pad)
PTW = 232
NST = 224             # matmul stream length
DVE_COLS = 138        # PSUM evacuation split: VectorE cols, rest ScalarE
CROWS = 28            # max rows per output SBUF chunk
CHUNK_SIZES = (4, 8, 16, 28, 28, 28, 28, 28, 28, 22)   # sums to 218
OIMG = KCH * HO * WO
HOWO = HO * WO

_NC_CACHE = {}


def make_w3(kernels: np.ndarray) -> np.ndarray:
    """Banded stationary matrices [128, 20*128].
    w3h[img*64 + dr*4 + g, (r*2+dd)*128 + img*64 + k] = kernels[k, dr-r, g+4dd]
    for 0 <= dr-r < 7 and g+4dd < 7, else 0."""
    w3 = np.zeros((10, 2, 128, 128), dtype=np.float32)
    for r in range(10):
        for dd in range(2):
            for dr in range(RB):
                di = dr - r
                if not (0 <= di < KS):
                    continue
                for g in range(4):
                    dj = g + 4 * dd
                    if dj >= KS:
                        continue
                    for img in range(2):
                        w3[r, dd, img * 64 + dr * 4 + g,
                           img * 64: img * 64 + KCH] = kernels[:, di, dj]
    return np.ascontiguousarray(
        w3.transpose(2, 0, 1, 3).reshape(128, 20 * 128))


def make_xs(xc: np.ndarray) -> np.ndarray:
    """Interleaved input segs [NQ, NSEG, 128, XSW] for one core's xc [4,H,W].
    xs[q, s, 2*p + img, j] = xc[2q+img, SEG_RLO[s]+p, j], zero-padded."""
    xs = np.zeros((NQ, NSEG, 128, XSW), dtype=np.float32)
    for s in range(NSEG):
        rlo = SEG_RLO[s]
        blk = xc[:, rlo: rlo + 64, :]            # [4, 64, 224]
        for img in range(2):
            xs[:, s, img::2, :W] = blk[img::2]   # [NQ, 64, 224]
    return xs


def _build_nc():
    import concourse.bacc as bacc
    import concourse.mybir as mybir
    import concourse.tile as tile
    from concourse.bass_types import AP

    F32 = mybir.dt.float32
    F32R = mybir.dt.float32r

    nc = bacc.Bacc("TRN2", target_bir_lowering=False, debug=False,
                   num_devices=NCORES)
    xs_d = nc.dram_tensor("xs", [NQ, NSEG, 128, XSW], F32R,
                          kind="ExternalInput").ap()
    w3_d = nc.dram_tensor("w3", [128, 20 * 128], F32R,
                          kind="ExternalInput").ap()
    out_d = nc.dram_tensor("out", [BLOC, KCH, HO, WO], F32,
                           kind="ExternalOutput").ap()

    with tile.TileContext(nc) as tc:
        with (
            tc.tile_pool(name="wpool", bufs=1) as wpool,
            tc.tile_pool(name="xspool", bufs=7) as xspool,
            tc.tile_pool(name="ptpool", bufs=6) as ptpool,
            tc.tile_pool(name="opool", bufs=3) as opool,
            tc.tile_pool(name="psum", bufs=8, space="PSUM") as psum,
        ):
            wfr = wpool.tile([128, 20 * 128], F32R)
            # seg s first needed at block 5s; prefetch ~3 blocks early
            load_before = {0: [0], 2: [1], 7: [2], 12: [3], 17: [4]}
            for q in range(NQ):
                xseg = [None] * NSEG
                gi = 0
                ci = 0          # chunk index within q
                chunk = None
                crow0 = 0
                nrows_c = 0
                for b in range(NBLK):
                    for s in load_before.get(b, ()):
                        xt_new = xspool.tile([128, XSW], F32R, tag="xs")
                        nc.sync.dma_start(out=xt_new[:], in_=xs_d[q, s])
                        xseg[s] = xt_new
                        if q == 0 and b == 0:
                            # W r=0,1 tiles first: unblocks block 0's
                            # matmuls; bulk follows behind gather b0.
                            nc.sync.dma_start(out=wfr[:, 0:512],
                                              in_=w3_d[:, 0:512])
                    r0 = NRB * b if b < NBLK - 1 else 208
                    s = SEG_OF_BLOCK[b]
                    xt = xseg[s]
                    pb = 2 * (r0 - SEG_RLO[s])   # partition of (row r0, img0)
                    pt = ptpool.tile([128, PTW], F32R, tag="pt")
                    for img in range(2):
                        nc.sync.dma_start(
                            out=pt[img * 64: (img + 1) * 64, :],
                            in_=AP(
                                tensor=xt[:].tensor,
                                offset=xt[:].offset + (pb + img) * XSW,
                                ap=((2 * XSW, RB), (1, 4), (1, PTW)),
                            ),
                        )
                    if q == 0 and b == 0:
                        nc.sync.dma_start(out=wfr[:, 512:2560],
                                          in_=w3_d[:, 512:2560])
                    rlist = range(NRB) if b < NBLK - 1 else range(2, NRB)
                    for r in rlist[::2]:
                        # two output rows share one PSUM bank
                        ps = psum.tile([128, 512], F32, tag="ps")
                        for p2 in range(2):
                            for dd in range(2):
                                t = ((r + p2) * 2 + dd) * 128
                                nc.tensor.matmul(
                                    out=ps[:, p2 * NST: (p2 + 1) * NST],
                                    lhsT=wfr[:, t: t + 128],
                                    rhs=pt[:, 4 * dd: 4 * dd + NST],
                                    start=(dd == 0), stop=(dd == 1),
                                )
                        slot = gi - crow0
                        if chunk is None:
                            nrows_c = CHUNK_SIZES[ci]
                            chunk = opool.tile([128, CROWS * WO], F32,
                                               tag="osb")
                        # 2-row evacuation, 3-dim APs
                        nc.vector.tensor_copy(
                            out=AP(tensor=chunk[:].tensor,
                                   offset=chunk[:].offset + slot * WO,
                                   ap=((CROWS * WO, 128), (WO, 2),
                                       (1, DVE_COLS))),
                            in_=AP(tensor=ps[:].tensor,
                                   offset=ps[:].offset,
                                   ap=((512, 128), (NST, 2),
                                       (1, DVE_COLS))))
                        nc.scalar.copy(
                            out=AP(tensor=chunk[:].tensor,
                                   offset=chunk[:].offset + slot * WO
                                   + DVE_COLS,
                                   ap=((CROWS * WO, 128), (WO, 2),
                                       (1, WO - DVE_COLS))),
                            in_=AP(tensor=ps[:].tensor,
                                   offset=ps[:].offset + DVE_COLS,
                                   ap=((512, 128), (NST, 2),
                                       (1, WO - DVE_COLS))))
                        if slot == nrows_c - 2:
                            st_in = AP(
                                tensor=chunk[:].tensor,
                                offset=chunk[:].offset,
                                ap=((CROWS * WO, 128), (1, nrows_c * WO)),
                            )
                            st_out = AP(
                                tensor=out_d.tensor,
                                offset=2 * q * OIMG + crow0 * WO,
                                ap=((OIMG, 2), (HOWO, KCH),
                                    (1, nrows_c * WO)),
                            )
                            nc.scalar.dma_start(out=st_out, in_=st_in)
                            chunk = None
                            ci += 1
                            crow0 = gi + 2
                        gi += 2
    nc.compile()
    return nc


def _get_nc():
    if "nc" not in _NC_CACHE:
        _NC_CACHE["nc"] = _build_nc()
    return _NC_CACHE["nc"]


def _run(x: np.ndarray, kernels: np.ndarray, **kw):
    from concourse.bass_utils import run_bass_kernel_spmd

    x = np.ascontiguousarray(np.asarray(x, dtype=np.float32))
    kernels = np.ascontiguousarray(np.asarray(kernels, dtype=np.float32))
    w3h = make_w3(kernels)
    nc = _get_nc()
    in_maps = [
        {"xs": make_xs(x[c * BLOC: (c + 1) * BLOC]), "w3": w3h}
        for c in range(NCORES)
    ]
    return run_bass_kernel_spmd(nc, in_maps, core_ids=list(range(NCORES)),
                                **kw)


def kernel(x: np.ndarray, kernels: np.ndarray) -> np.ndarray:
    res = _run(x, kernels)
    return np.concatenate([res.results[c]["out"] for c in range(NCORES)],
                          axis=0)
